# revision 46
# baseline (speedup 1.0000x reference)
"""DeepTDATransformer TRN2 Bass kernel: 4-core sample-parallel, v2.

Key optimizations over baseline:
- Head-pair packed attention: scores at 2 row-groups + attV at 2 col-groups
  overlap on the PE array; exp done as [128,1024] ACTs with ts folded into
  the activation scale (removes Q pre-scaling and the PH->Q dependency).
- No Ln activations anywhere (Ln/Exp live in different ACT table sets and
  every alternation costs a 1.3us table load): rsqrt = Sqrt + DVE
  reciprocal_approx_fast; softmax normalize via reciprocal_approx_fast.
- LayerNorm as outer products: A = g (x) rstd, B = b (x) 1 + g (x) (-m*rstd)
  built by rank-1 matmuls, applied with 2 DVE passes.
- PSUM retagged to sc(2 banks x2)/ab(1 bank x2)/pj(1 bank x2) = 8 banks.
- Double-buffered weight DMA (wp pool bufs=2), QKV of layer l+1 issued in
  layer l's tail.
"""
import numpy as np
import concourse.bacc as bacc
import concourse.tile as tile
import concourse.mybir as mybir
from concourse import bass_utils

dt = mybir.dt
AF = mybir.ActivationFunctionType
ALU = mybir.AluOpType
AX = mybir.AxisListType
F32 = np.float32
TS = dt.float32
TR = dt.float32r
TB = dt.bfloat16

S, SP, E, H, DH, L, NCls = 1000, 1024, 256, 8, 32, 6, 2
EC = 2   # e chunks
HC = 8   # ffn hidden chunks

_uid = [0]


def _nm(p="i"):
    _uid[0] += 1
    return f"{p}{_uid[0]}"


def build_nc():
    nc = bacc.Bacc("TRN2", target_bir_lowering=False, debug=False, num_devices=4)
    d = {}

    def din(name, shape):
        d[name] = nc.dram_tensor(name, list(shape), dt.float32, kind="ExternalInput").ap()

    din("seqT5", (5, SP)); din("seqPH", (128, 40))
    din("embw1T", (5, 128)); din("embb1", (128, 1))
    din("embw2T", (128, EC * 128)); din("embb2", (128, EC))
    din("posT", (128, EC * SP))
    din("qwT", (128, L * EC * E)); din("kwT", (128, L * EC * E))
    din("vwT", (128, L * EC * E)); din("owT", (128, L * EC * E))
    din("qkvb", (128, L * 3 * EC)); din("obias", (128, L * EC))
    din("vbrow", (1, L * E))
    din("fw1T", (128, L * EC * 1024)); din("fw2T", (128, L * HC * E))
    din("fb1", (128, L * HC)); din("fb2", (128, L * EC))
    din("lng", (128, L * EC)); din("lnb", (128, L * EC))
    din("embln", (128, 2 * EC))
    din("pew1T", (6, L * 128)); din("peb1", (128, L))
    din("pew2T", (128, L * E)); din("peb2", (128, L * EC))
    din("clng", (128, EC)); din("clnb", (128, EC))
    din("cw1T", (128, EC * 128)); din("cb1", (128, 1))
    din("cw2T", (128, NCls)); din("cb2", (NCls, 1))
    din("ph_law", (1, 2)); din("ph_lab", (1, 1)); din("ph_fw", (1, 1)); din("ph_db", (1, 1))
    din("tconst", (128, 8)); din("padneg", (128, 8))
    din("iota50", (128, 50)); din("I50", (50, 50)); din("maskD50", (50, 50)); din("I4", (4, 4))
    din("I128", (128, 128)); din("ones128", (128, 1)); din("ones50", (50, 1))
    din("ones1x128", (1, 128)); din("ones1x50", (1, 50)); din("ones1x32", (1, 32))
    din("ones1x512", (1, 512))
    din("ones4", (4, 1)); din("onesEC", (EC, 1)); din("oneso256", (128, 1))
    din("v0", (50, 1)); din("W0", (50, 4)); din("zeros128", (128, 256))
    din("epsb", (128, 1))
    din("vmask8", (128, 64))
    out_d = nc.dram_tensor("out", [NCls, 1], dt.float32, kind="ExternalOutput").ap()
    import os
    DBG = os.environ.get("KDBG", "") == "1"
    if DBG:
        dbg_d = nc.dram_tensor("dbg", [128, 8192], dt.float32, kind="ExternalOutput").ap()

    with tile.TileContext(nc) as tc:
        with (
            tc.tile_pool(name="const", bufs=1) as cp,
            tc.tile_pool(name="wp", bufs=2) as wp,
            tc.tile_pool(name="ap_", bufs=1) as app,
            tc.tile_pool(name="sm", bufs=1) as sm,
            tc.tile_pool(name="sm4", bufs=2) as sm4,
            tc.tile_pool(name="ps", bufs=2, space="PSUM") as ps,
        ):
            def c32(name, shape, nm=None):
                t = cp.tile(list(shape), TS, tag=nm or name, name=nm or name)
                nc.sync.dma_start(t[:], d[name])
                return t

            def c32r(name, shape, nm=None):
                t = cp.tile(list(shape), TR, tag=(nm or name) + "r", name=(nm or name) + "r")
                nc.gpsimd.dma_start(t[:], d[name])
                return t

            seqT5 = c32r("seqT5", (5, SP))
            seqPH = c32("seqPH", (128, 40))
            tconst = c32("tconst", (128, 8)); padneg = c32("padneg", (128, 8))
            iota50 = c32("iota50", (128, 50))
            I50r = c32r("I50", (50, 50)); maskD50 = c32("maskD50", (50, 50))
            I4 = c32("I4", (4, 4)); I4r = c32r("I4", (4, 4), "I4c"); I128r = c32r("I128", (128, 128))
            ones128r = c32r("ones128", (128, 1)); ones50r = c32r("ones50", (50, 1))
            o1x128r = c32r("ones1x128", (1, 128)); o1x50r = c32r("ones1x50", (1, 50))
            o1x32r = c32r("ones1x32", (1, 32)); ones4r = c32r("ones4", (4, 1))
            o1x512r = c32r("ones1x512", (1, 512))
            onesECr = c32r("onesEC", (EC, 1)); oneso256 = c32r("oneso256", (128, 1))
            v0 = c32r("v0", (50, 1)); W0r = c32r("W0", (50, 4))
            zeros128 = c32("zeros128", (128, 256))
            vmask8 = c32("vmask8", (128, 64))
            epsb = c32("epsb", (128, 1))
            embw1T = c32r("embw1T", (5, 128)); embb1 = c32("embb1", (128, 1))
            embw2T = c32r("embw2T", (128, EC * 128)); embb2 = c32("embb2", (128, EC))
            posT = app.tile([128, EC * SP], TS, tag="att", bufs=1, name="posT")
            nc.sync.dma_start(posT[:], d["posT"])
            lng = c32("lng", (128, L * EC)); lnb = c32("lnb", (128, L * EC))
            embln = c32("embln", (128, 2 * EC))
            pew1T = c32r("pew1T", (6, L * 128)); peb1 = c32("peb1", (128, L))
            pew2T = c32r("pew2T", (128, L * E)); peb2 = c32("peb2", (128, L * EC))
            clng = c32("clng", (128, EC)); clnb = c32("clnb", (128, EC))
            cw1T = c32r("cw1T", (128, EC * 128)); cb1 = c32("cb1", (128, 1))
            cw2T = c32r("cw2T", (128, NCls)); cb2 = c32("cb2", (NCls, 1))
            law = c32r("ph_law", (1, 2)); lab = c32r("ph_lab", (1, 1))
            phfw = c32r("ph_fw", (1, 1)); phdb = c32r("ph_db", (1, 1))

            def pt(shape, tag="pj"):
                # psum: tags sc([128,1024] 2 banks x2), ab([128,512] x2), pj([128,512] x2)
                return ps.tile(list(shape), TS, tag=tag, name=_nm("p"))

            def sb(shape, dtype=TS, pool=sm, tag=None):
                if tag is None:
                    fbytes = int(np.prod(shape[1:])) * 4
                    if fbytes >= 2048:
                        return sm4.tile(list(shape), dtype, tag=f"g{fbytes}", name=_nm("s"))
                    tag = _nm("t")
                return pool.tile(list(shape), dtype, tag=tag, name=_nm("s"))

            def copy(dst, src):
                nc.vector.tensor_copy(dst, src)

            MM = nc.tensor.matmul

            def MMs(out, lhsT, rhs, **kw):
                l2 = lhsT.bitcast(TS) if lhsT.dtype == TR else lhsT
                r2 = rhs.bitcast(TS) if rhs.dtype == TR else rhs
                return MM(out, l2, r2, **kw)

            def rsqrt_row(dst_ts, src_ap, scale=1.0, bias=None):
                # dst = 1/sqrt(scale*src + bias); dst/src any [1,n] or [p,n]
                shp = [dst_ts.partition_size(), dst_ts.free_size()]
                tmp = sm.tile([1, 512], TS, tag="rstmp", name=_nm("s"))
                tv = tmp[0:shp[0], 0:shp[1]]
                if bias is None:
                    nc.scalar.activation(tv, src_ap, AF.Sqrt, scale=scale)
                else:
                    nc.scalar.activation(tv, src_ap, AF.Sqrt, scale=scale, bias=bias)
                nc.vector.reciprocal_approx_fast(dst_ts, tv)

            # ================= PH =================
            mfeat = sb((128, 8))
            nc.vector.tensor_reduce(mfeat[:], seqPH[:].rearrange("p (c f) -> p c f", f=5),
                                    AX.X, ALU.add)
            nc.vector.tensor_scalar_mul(mfeat[:], mfeat[:], 0.2)
            p1 = pt((128, 8))
            MMs(p1[:, 0:2], o1x128r[:], law[:], start=True, stop=True)
            MMs(p1[:, 2:3], o1x128r[:], lab[:], start=True, stop=True)
            lawB = sb((128, 4))
            copy(lawB[:], p1[:, 0:4])
            scs = sb((128, 8))
            nc.vector.tensor_scalar(scs[:], tconst[:], lawB[:, 0:1], None, ALU.mult)
            tmp8 = sb((128, 8))
            nc.vector.tensor_scalar(tmp8[:], mfeat[:], lawB[:, 1:2], None, ALU.mult)
            nc.vector.tensor_add(scs[:], scs[:], tmp8[:])
            nc.vector.tensor_scalar(scs[:], scs[:], lawB[:, 2:3], None, ALU.add)
            nc.vector.tensor_add(scs[:], scs[:], padneg[:])
            scr = sb((128, 8), TR)
            copy(scr[:], scs[:])
            p2 = pt((1, 1024), tag="sc")
            for c in range(8):
                MMs(p2[:, c * 128:(c + 1) * 128], scr[:, c:c + 1], I128r[:], start=True, stop=True)
            srow = sb((1, 1024), TR)
            copy(srow[:], p2[:])
            sROW = sb((128, 1024))
            for hh in range(2):
                p3 = pt((128, 512))
                MMs(p3[:], o1x128r[:], srow[:, hh * 512:(hh + 1) * 512],
                    start=True, stop=True)
                copy(sROW[:, hh * 512:(hh + 1) * 512], p3[:])
            rank = sb((128, 8))
            scratch = sb((128, 1024))
            for c in range(8):
                nc.vector.tensor_scalar(scratch[:], sROW[:], scs[:, c:c + 1], 0.0,
                                        ALU.is_gt, ALU.add, accum_out=rank[:, c:c + 1])
            ptsr = sb((128, 16), TR)
            pv = ptsr[:].rearrange("p (c two) -> p c two", two=2)
            copy(pv[:, :, 0:1], tconst[:].rearrange("p (c o) -> p c o", o=1))
            copy(pv[:, :, 1:2], mfeat[:].rearrange("p (c o) -> p c o", o=1))
            Gc = sb((128, 400), TR, tag="Gc", pool=app)
            for c in range(8):
                nc.vector.tensor_scalar(Gc[:, c * 50:(c + 1) * 50], iota50[:],
                                        rank[:, c:c + 1], None, ALU.is_equal)
            plmT = pt((2, 50), tag="ab")
            for c in range(8):
                MMs(plmT[:], ptsr[:, c * 2:(c + 1) * 2], Gc[:, c * 50:(c + 1) * 50],
                   start=(c == 0), stop=(c == 7))
            lmT = sb((2, 50), TR)
            copy(lmT[:], plmT[:])
            pg = pt((50, 50))
            MMs(pg[:], lmT[:], lmT[:], start=True, stop=True)
            gram = sb((50, 50))
            copy(gram[:], pg[:])
            sqd = sb((50, 50))
            nc.vector.tensor_mul(sqd[:], gram[:], maskD50[:])
            sq = sb((50, 1))
            nc.vector.tensor_reduce(sq[:], sqd[:], AX.X, ALU.add)
            t1 = sb((50, 50))
            nc.vector.tensor_scalar(t1[:], gram[:], -2.0, sq[:], ALU.mult, ALU.add)
            sqr = sb((50, 1), TR)
            copy(sqr[:], sq[:])
            p4 = pt((1, 50), tag="ab")
            MMs(p4[:], sqr[:], I50r[:], start=True, stop=True)
            sqrow = sb((1, 50), TR)
            copy(sqrow[:], p4[:])
            p5 = pt((50, 50), tag="ab")
            MMs(p5[:], o1x50r[:], sqrow[:], start=True, stop=True)
            d2 = sb((50, 50))
            nc.vector.tensor_add(d2[:], t1[:], p5[:])
            nc.vector.tensor_scalar_max(d2[:], d2[:], 1e-30)
            distm = sb((50, 50))
            nc.scalar.activation(distm[:], d2[:], AF.Sqrt)
            p6 = pt((50, 2), tag="pj")
            MMs(p6[:, 0:1], o1x50r[:], phfw[:], start=True, stop=True)
            MMs(p6[:, 1:2], o1x50r[:], phdb[:], start=True, stop=True)
            fwdb = sb((50, 2))
            copy(fwdb[:], p6[:])
            nfw = sb((50, 2))
            nc.scalar.activation(nfw[:, 0:1], fwdb[:, 0:1], AF.Abs)
            nc.vector.tensor_scalar_mul(nfw[:, 1:2], fwdb[:, 1:2], -1.0)
            dists = sb((50, 50))
            nc.vector.tensor_scalar(dists[:], distm[:], nfw[:, 0:1], None, ALU.mult)
            Km = sb((50, 50))
            nc.scalar.activation(Km[:], dists[:], AF.Exp, scale=-1.0, bias=nfw[:, 1:2])
            s_r = sb((50, 1))
            nc.vector.tensor_reduce(s_r[:], Km[:], AX.X, ALU.add)
            Bm = sb((50, 50))
            nc.vector.tensor_scalar(Bm[:], maskD50[:], s_r[:], None, ALU.mult)
            nc.vector.tensor_sub(Bm[:], Bm[:], Km[:])
            nc.vector.tensor_scalar_mul(Bm[:], Bm[:], -1.0)
            D40 = sb((50, 50))
            nc.vector.tensor_scalar_mul(D40[:], maskD50[:], 40.0)
            nc.vector.tensor_add(Bm[:], Bm[:], D40[:])
            nc.vector.tensor_scalar_add(Bm[:], Bm[:], -0.8)
            Br = sb((50, 50), TR)
            copy(Br[:], Bm[:])

            def vec_norm(vr):
                pn = pt((1, 1))
                MMs(pn[:], vr[:], vr[:], start=True, stop=True)
                rs = sb((1, 1), TR)
                rsqrt_row(rs[:].bitcast(TS), pn[:])
                prb = pt((50, 1), tag="pj")
                MMs(prb[:], o1x50r[:], rs[:], start=True, stop=True)
                vn = sm.tile([50, 1], TR, tag="vpow", bufs=2, name=_nm("s"))
                nc.vector.tensor_mul(vn[:].bitcast(TS), vr[:].bitcast(TS), prb[:])
                vn2 = sm.tile([50, 1], TR, tag="vpow", bufs=2, name=_nm("s"))
                copy(vn2[:], vn[:].bitcast(TS))
                return vn2

            v = v0
            for it in range(12):
                pv_ = pt((50, 1))
                MMs(pv_[:], Br[:], v[:], start=True, stop=True)
                v = sm.tile([50, 1], TR, tag="vpow", bufs=2, name=_nm("s"))
                nc.vector.tensor_scalar_mul(v[:], pv_[:], 0.125)
                if it % 4 == 3:
                    v = vec_norm(v)
            v = vec_norm(v)
            pbv = pt((50, 1))
            MMs(pbv[:], Br[:], v[:], start=True, stop=True)
            vbvf = sb((50, 1))
            nc.vector.tensor_mul(vbvf[:], v[:].bitcast(TS), pbv[:])
            vbv = sb((50, 1), TR)
            copy(vbv[:], vbvf[:])
            pmu = pt((1, 1))
            MMs(pmu[:], vbv[:], ones50r[:], start=True, stop=True)
            mu1 = sb((1, 1))
            copy(mu1[:], pmu[:])
            pvr = pt((1, 50), tag="ab")
            MMs(pvr[:], v[:], I50r[:], start=True, stop=True)
            vRow = sb((1, 50), TR)
            copy(vRow[:], pvr[:])

            def ns_orth(W, nstep):
                pg_ = pt((4, 4))
                MMs(pg_[:], W[:], W[:], start=True, stop=True)
                gd = sb((4, 4))
                nc.vector.tensor_mul(gd[:], pg_[:], I4[:])
                gdr = sb((4, 1))
                nc.vector.tensor_reduce(gdr[:], gd[:], AX.X, ALU.add)
                gdr2 = sb((4, 1), TR)
                copy(gdr2[:], gdr[:])
                ptr = pt((1, 1))
                MMs(ptr[:], gdr2[:], ones4r[:], start=True, stop=True)
                rst = sb((1, 1), TR)
                rsqrt_row(rst[:].bitcast(TS), ptr[:], scale=0.25)
                prb = pt((50, 1), tag="pj")
                MMs(prb[:], o1x50r[:], rst[:], start=True, stop=True)
                Wn = sm.tile([50, 4], TR, tag="Wsub", bufs=2, name=_nm("s"))
                nc.vector.tensor_scalar(Wn[:], W[:].bitcast(TS), prb[:], None, ALU.mult)
                W = Wn
                for _ in range(nstep):
                    pg2 = pt((4, 4))
                    MMs(pg2[:], W[:], W[:], start=True, stop=True)
                    i4h = sb((4, 4))
                    nc.vector.tensor_scalar_mul(i4h[:], I4[:], 1.5)
                    corrf = sb((4, 4))
                    nc.vector.tensor_scalar(corrf[:], pg2[:], -0.5, None, ALU.mult)
                    corr = sb((4, 4), TR)
                    nc.vector.tensor_add(corr[:], corrf[:], i4h[:])
                    pwt = pt((4, 50), tag="ab")
                    MMs(pwt[:], W[:], I50r[:], start=True, stop=True)
                    WT = sb((4, 50), TR)
                    copy(WT[:], pwt[:])
                    pw2 = pt((50, 4), tag="pj")
                    MMs(pw2[:], WT[:], corr[:], start=True, stop=True)
                    W = sm.tile([50, 4], TR, tag="Wsub", bufs=2, name=_nm("s"))
                    copy(W[:], pw2[:])
                return W

            W = W0r
            for it in range(14):
                pw_ = pt((50, 4))
                MMs(pw_[:], Br[:], W[:], start=True, stop=True)
                Wn = sm.tile([50, 4], TR, tag="Wsub", bufs=2, name=_nm("s"))
                nc.vector.tensor_scalar_mul(Wn[:], pw_[:], 0.125)
                W = Wn
                pc_ = pt((1, 4))
                MMs(pc_[:], v[:], W[:], start=True, stop=True)
                cvw = sb((1, 4), TR)
                copy(cvw[:], pc_[:])
                pcor = pt((50, 4), tag="pj")
                MMs(pcor[:], vRow[:], cvw[:], start=True, stop=True)
                Wn = sm.tile([50, 4], TR, tag="Wsub", bufs=2, name=_nm("s"))
                nc.vector.tensor_sub(Wn[:].bitcast(TS), W[:].bitcast(TS), pcor[:])
                W2_ = sm.tile([50, 4], TR, tag="Wsub", bufs=2, name=_nm("s"))
                copy(W2_[:], Wn[:].bitcast(TS))
                W = W2_
                if it % 6 == 5:
                    W = ns_orth(W, 3)
            W = ns_orth(W, 6)
            pbw = pt((50, 4))
            MMs(pbw[:], Br[:], W[:], start=True, stop=True)
            BW = sb((50, 4), TR)
            copy(BW[:], pbw[:])
            ph4 = pt((4, 4))
            MMs(ph4[:], W[:], BW[:], start=True, stop=True)
            H4 = sb((4, 4))
            copy(H4[:], ph4[:])
            h4d = sb((4, 4)); h4f = sb((4, 4))
            nc.vector.tensor_mul(h4d[:], H4[:], I4[:])
            nc.vector.tensor_mul(h4f[:], H4[:], H4[:])
            rd = sb((4, 1)); rf = sb((4, 1))
            nc.vector.tensor_reduce(rd[:], h4d[:], AX.X, ALU.add)
            nc.vector.tensor_reduce(rf[:], h4f[:], AX.X, ALU.add)
            rdr = sb((4, 2), TR)
            copy(rdr[:, 0:1], rd[:]); copy(rdr[:, 1:2], rf[:])
            pst = pt((2, 1))
            MMs(pst[:], rdr[:], ones4r[:], start=True, stop=True)
            stt2 = sb((2, 1), TR)
            copy(stt2[:], pst[:])
            pstr = pt((1, 2))
            MMs(pstr[:], stt2[:], I4r[0:2, 0:2], start=True, stop=True)
            sttrow = sb((1, 2))
            copy(sttrow[:], pstr[:])
            frH0 = sttrow[0:1, 1:2]
            mean_mu = sb((1, 1))
            nc.vector.tensor_scalar_mul(mean_mu[:], sttrow[0:1, 0:1], 0.25)
            m2 = sb((1, 1))
            nc.vector.tensor_mul(m2[:], mean_mu[:], mean_mu[:])
            nc.vector.tensor_scalar_mul(m2[:], m2[:], -4.0 / 3.0)
            varq = sb((1, 1))
            nc.vector.tensor_scalar_mul(varq[:], frH0[:], 1.0 / 3.0)
            nc.vector.tensor_add(varq[:], varq[:], m2[:])
            nc.vector.tensor_scalar_max(varq[:], varq[:], 1e-30)
            std_ev = sb((1, 1))
            nc.scalar.activation(std_ev[:], varq[:], AF.Sqrt)
            mean_ev = sb((1, 1))
            nc.vector.tensor_scalar(mean_ev[:], mean_mu[:], -1.0, 40.0, ALU.mult, ALU.add)
            gap = sb((1, 1))
            nc.vector.tensor_scalar(gap[:], mu1[:], -1.0, 40.0, ALU.mult, ALU.add)
            pfrow = sb((1, 8))
            copy(pfrow[:], zeros128[0:1, 0:8])
            nc.vector.tensor_scalar_add(pfrow[:, 0:1], pfrow[:, 0:1], 1.0)
            nc.vector.tensor_scalar_add(pfrow[:, 3:4], pfrow[:, 3:4], 1.0 / 7.0)
            copy(pfrow[:, 2:3], gap[:])
            copy(pfrow[:, 4:5], mean_ev[:])
            copy(pfrow[:, 5:6], std_ev[:])
            pfrr = sb((1, 8), TR)
            copy(pfrr[:], pfrow[:])
            ppf = pt((8, 1))
            MMs(ppf[:], pfrr[:], o1x128r[:, 0:1], start=True, stop=True)
            pfr = sb((8, 1), TR)
            copy(pfr[:], ppf[:])
            pfr = pfr[0:6, :]

            # ts per layer
            tsB = sb((128, L), tag="tsB", pool=app)
            for l in range(L):
                ph1 = pt((128, 1))
                MMs(ph1[:], pew1T[:, l * 128:(l + 1) * 128], pfr[:], start=True, stop=True)
                h1f = sb((128, 1))
                nc.vector.tensor_scalar(h1f[:], ph1[:], peb1[:, l:l + 1], None, ALU.add)
                h1b = sb((128, 1), TR)
                nc.vector.tensor_scalar_max(h1b[:], h1f[:], 0.0)
                sig = sb((128, EC))
                for co in range(EC):
                    py = pt((128, 1))
                    MMs(py[:], pew2T[:, (l * EC + co) * 128:(l * EC + co + 1) * 128],
                       h1b[:], start=True, stop=True)
                    yb = sb((128, 1))
                    nc.vector.tensor_scalar(yb[:], py[:], peb2[:, l * EC + co:l * EC + co + 1],
                                            None, ALU.add)
                    ey = sb((128, 1))
                    nc.scalar.activation(ey[:], yb[:], AF.Exp, scale=-1.0)
                    nc.vector.tensor_scalar_add(ey[:], ey[:], 1.0)
                    nc.vector.reciprocal(sig[:, co:co + 1], ey[:])
                sigr = sb((128, EC), TR)
                copy(sigr[:], sig[:])
                pts_ = pt((EC, 1))
                MMs(pts_[:], sigr[:], ones128r[:], start=True, stop=True)
                tsum = sb((EC, 1), TR)
                copy(tsum[:], pts_[:])
                pt2 = pt((1, 1))
                MMs(pt2[:], tsum[:], onesECr[:], start=True, stop=True)
                tsv = sb((1, 1), TR)
                nc.vector.tensor_scalar_mul(tsv[:], pt2[:], float(1.0 / (256.0 * np.sqrt(32.0))))
                ptb = pt((128, 1))
                MMs(ptb[:], o1x128r[:], tsv[:], start=True, stop=True)
                copy(tsB[:, l:l + 1], ptb[:])

            # ================= embedding =================
            e1 = sb((128, SP), TR)
            for th in range(2):
                pe_ = pt((128, 512))
                MM(pe_[:, 0:500], embw1T[:], seqT5[:, th * 500:(th + 1) * 500], start=True, stop=True)
                nc.vector.tensor_scalar(e1[:, th * 500:(th + 1) * 500], pe_[:, 0:500],
                                        embb1[:], None, ALU.add)
            e1r = sb((128, SP), TR)
            nc.vector.tensor_scalar_max(e1r[:], e1[:].bitcast(TS), 0.0)
            xemb = sb((128, EC * SP), TR, tag="resid", pool=app)
            for co in range(EC):
                for th in range(2):
                    px = pt((128, 512))
                    MM(px[:, 0:500], embw2T[:, co * 128:(co + 1) * 128],
                       e1r[:, th * 500:th * 500 + 500], start=True, stop=True)
                    nc.vector.tensor_scalar(xemb[:, co * SP + th * 500: co * SP + (th + 1) * 500],
                                            px[:, 0:500], embb2[:, co:co + 1], None, ALU.add)
            for co in range(EC):
                copy(xemb[:, co * SP + 1000: co * SP + 1024], zeros128[:, 0:24])

            # --------- LayerNorm via outer products ---------
            # out = x*A + B, A = g (x) rstd, B = b (x) 1 + g (x) (-m*rstd)
            def ln_T(xin, gcol, bcol, add_pos=False):
                # stats rows: cols th*512 .. th*512+500 hold tokens th*500..
                rmean = sm.tile([1, 1024], TS, tag="rmean", bufs=1, name=_nm("s"))
                rvar = sm.tile([1, 1024], TS, tag="rvar", bufs=1, name=_nm("s"))
                rstd = sm.tile([1, 1024], TR, tag="rstd", bufs=1, name=_nm("s"))
                rmrs = sm.tile([1, 1024], TR, tag="rmrs", bufs=1, name=_nm("s"))
                for th in range(2):
                    x2 = sm4.tile([128, 1024], TR, tag="x2t", bufs=1, name=_nm("s"))
                    for co in range(EC):
                        sl = xin[:, co * SP + th * 500: co * SP + (th + 1) * 500]
                        nc.vector.tensor_mul(x2[:, co * 512:co * 512 + 500],
                                             sl.bitcast(TS), sl.bitcast(TS))
                    stm = pt((1, 512))
                    for co in range(EC):
                        MM(stm[0:1, 0:500], oneso256[:],
                           xin[:, co * SP + th * 500: co * SP + (th + 1) * 500],
                           start=(co == 0), stop=(co == EC - 1))
                    ste = pt((1, 512))
                    for co in range(EC):
                        MM(ste[0:1, 0:500], oneso256[:], x2[:, co * 512:co * 512 + 500],
                           start=(co == 0), stop=(co == EC - 1))
                    # rows: m2 -> var+eps -> 1/x (DVE) -> sqrt (ACT, rounds to f32r)
                    mslice = rmean[0:1, th * 512: th * 512 + 500]
                    copy(mslice, stm[0:1, 0:500])
                    vsl = rvar[0:1, th * 512: th * 512 + 500]
                    nc.vector.tensor_mul(vsl, mslice, mslice)
                    nc.vector.scalar_tensor_tensor(vsl, ste[0:1, 0:500], 1e-5,
                                                   vsl, ALU.add, ALU.subtract)
                    rec = sm.tile([1, 512], TS, tag="rstmp", name=_nm("s"))
                    nc.vector.reciprocal_approx_fast(rec[0:1, 0:500], vsl)
                    ssl = rstd[0:1, th * 512: th * 512 + 500]
                    nc.scalar.activation(ssl, rec[0:1, 0:500], AF.Sqrt)
                    nc.vector.scalar_tensor_tensor(
                        rmrs[0:1, th * 512: th * 512 + 500],
                        mslice, -1.0, ssl.bitcast(TS), ALU.mult, ALU.mult)
                out = app.tile([128, EC * SP], TR, tag="x_ln", bufs=1, name=_nm("s"))
                for th in range(2):
                    pR = pt((128, 512))
                    MM(pR[:], o1x128r[:], rstd[0:1, th * 512:(th + 1) * 512], start=True, stop=True)
                    pM = pt((128, 512))
                    MM(pM[:], o1x128r[:], rmrs[0:1, th * 512:(th + 1) * 512], start=True, stop=True)
                    for co in range(EC):
                        tmp = sm4.tile([128, 512], TS, tag="lntmp", bufs=1, name=_nm("s"))
                        tmp2 = sm4.tile([128, 512], TS, tag="lntmp2", bufs=1, name=_nm("s"))
                        sl = xin[:, co * SP + th * 500: co * SP + (th + 1) * 500]
                        nc.vector.tensor_mul(tmp[:, 0:500], sl.bitcast(TS), pR[:, 0:500])
                        nc.vector.tensor_add(tmp2[:, 0:500], tmp[:, 0:500], pM[:, 0:500])
                        osl = out[:, co * SP + th * 500: co * SP + (th + 1) * 500]
                        if add_pos:
                            nc.vector.tensor_scalar(tmp[:, 0:500], tmp2[:, 0:500],
                                                    gcol[:, co:co + 1], bcol[:, co:co + 1],
                                                    ALU.mult, ALU.add)
                            nc.vector.tensor_add(osl, tmp[:, 0:500],
                                                 posT[:, co * SP + th * 500: co * SP + (th + 1) * 500])
                        else:
                            nc.vector.tensor_scalar(osl, tmp2[:, 0:500],
                                                    gcol[:, co:co + 1], bcol[:, co:co + 1],
                                                    ALU.mult, ALU.add)
                for co in range(EC):
                    copy(out[:, co * SP + 1000: co * SP + 1024], zeros128[:, 0:24])
                return out

            x = ln_T(xemb, embln[:, 0:EC], embln[:, EC:2 * EC], add_pos=True)
            if DBG:
                nc.sync.dma_start(dbg_d[:, 0:2048], x[:].bitcast(TS))

            # ---------- per-layer weight loading ----------
            wtiles = {}

            def load_qkv_w(l):
                if l >= L:
                    return
                t = {}
                t["wq"] = wp.tile([128, EC * E], TR, tag="wq", name=_nm("wq"))
                nc.gpsimd.dma_start(t["wq"][:], d["qwT"][:, l * EC * E:(l + 1) * EC * E])
                t["wk"] = wp.tile([128, EC * E], TR, tag="wk", name=_nm("wk"))
                nc.gpsimd.dma_start(t["wk"][:], d["kwT"][:, l * EC * E:(l + 1) * EC * E])
                t["wv"] = wp.tile([128, EC * E], TR, tag="wv", name=_nm("wv"))
                nc.gpsimd.dma_start(t["wv"][:], d["vwT"][:, l * EC * E:(l + 1) * EC * E])
                t["vbr"] = wp.tile([1, E], TR, tag="vbr", name=_nm("vbr"))
                nc.gpsimd.dma_start(t["vbr"][:], d["vbrow"][:, l * E:(l + 1) * E])
                t["bq"] = wp.tile([128, 3 * EC], TS, tag="bqkv", name=_nm("bq"))
                nc.sync.dma_start(t["bq"][:], d["qkvb"][:, l * 3 * EC:(l + 1) * 3 * EC])
                wtiles[("qkv", l)] = t

            def load_tail_w(l):
                t = {}
                t["wo"] = wp.tile([128, EC * E], TR, tag="wo", name=_nm("wo"))
                nc.gpsimd.dma_start(t["wo"][:], d["owT"][:, l * EC * E:(l + 1) * EC * E])
                t["w1"] = wp.tile([128, EC * 1024], TR, tag="w1", name=_nm("w1"))
                nc.gpsimd.dma_start(t["w1"][:], d["fw1T"][:, l * EC * 1024:(l + 1) * EC * 1024])
                t["w2"] = wp.tile([128, HC * E], TR, tag="w2", name=_nm("w2"))
                nc.gpsimd.dma_start(t["w2"][:], d["fw2T"][:, l * HC * E:(l + 1) * HC * E])
                t["bo"] = wp.tile([128, EC], TS, tag="bo", name=_nm("bo"))
                nc.sync.dma_start(t["bo"][:], d["obias"][:, l * EC:(l + 1) * EC])
                t["b1"] = wp.tile([128, HC], TS, tag="b1", name=_nm("b1"))
                nc.sync.dma_start(t["b1"][:], d["fb1"][:, l * HC:(l + 1) * HC])
                t["b2"] = wp.tile([128, EC], TS, tag="b2", name=_nm("b2"))
                nc.sync.dma_start(t["b2"][:], d["fb2"][:, l * EC:(l + 1) * EC])
                wtiles[("tail", l)] = t

            # ---------- QKV + V build for a layer ----------
            def qkv_issue(l, x):
                t = wtiles.pop(("qkv", l))
                wq, wk, wv, vbr, bq = t["wq"], t["wk"], t["wv"], t["vbr"], t["bq"]
                qTs = sb((128, EC * SP), TB, tag="qTs", pool=app)
                kT = sb((128, EC * SP), TB, tag="kT", pool=app)
                for (wt, outt, bofs, scl) in ((wq, qTs, 0, True), (wk, kT, EC, False)):
                    for co in range(EC):
                        for th in range(2):
                            pp = pt((128, 512))
                            for ci in range(EC):
                                MM(pp[:, 0:500],
                                   wt[:, (ci * EC + co) * 128:(ci * EC + co + 1) * 128],
                                   x[:, ci * SP + th * 500: ci * SP + (th + 1) * 500],
                                   start=(ci == 0), stop=(ci == EC - 1))
                            osl = outt[:, co * SP + th * 500: co * SP + (th + 1) * 500]
                            if scl:
                                nc.vector.tensor_scalar(
                                    osl, pp[:, 0:500], bq[:, bofs + co: bofs + co + 1],
                                    tsB[:, l:l + 1], ALU.add, ALU.mult)
                            else:
                                nc.vector.tensor_scalar(
                                    osl, pp[:, 0:500], bq[:, bofs + co: bofs + co + 1],
                                    None, ALU.add)
                    for co in range(EC):
                        copy(outt[:, co * SP + 1000: co * SP + 1024], zeros128[:, 0:24])
                # V token-major: Vtm [128, tc*(H*33)]
                Vtm = sb((128, 8 * 264), TB, tag="Vtm", pool=app)
                vslice = Vtm[:].rearrange("p (tc h c) -> p tc h c", tc=8, h=H)
                for tcb in range(8):
                    copy(vslice[:, tcb, :, 32:33],
                         vmask8[:, tcb * 8:(tcb + 1) * 8].rearrange("p (h o) -> p h o", o=1))
                nc.vector.tensor_copy(
                    vslice[96:128, 7, :, 0:32],
                    zeros128[0:32, 0:256].rearrange("p (h dd) -> p h dd", h=H))
                for tcb in range(8):
                    pv2 = pt((128, 512))
                    for ci in range(EC):
                        MM(pv2[:, 0:256],
                           x[:, ci * SP + tcb * 128: ci * SP + (tcb + 1) * 128],
                           wv[:, ci * E:(ci + 1) * E],
                           start=(ci == 0), stop=False)
                    MMs(pv2[:, 0:256], o1x128r[:], vbr[:], start=False, stop=True)
                    nrows = 128 if tcb < 7 else 104
                    nc.vector.tensor_copy(
                        vslice[0:nrows, tcb, :, 0:32],
                        pv2[0:nrows, 0:256].rearrange("p (h dd) -> p h dd", h=H))
                return qTs, kT, Vtm

            load_qkv_w(0)
            load_tail_w(0)
            load_qkv_w(1)
            qTs, kT, Vtm = qkv_issue(0, x)

            # ================= layers =================
            for l in range(L):
                if l + 1 < L:
                    load_tail_w(l + 1)
                    load_qkv_w(l + 2)
                # ---- attention ----
                att = sb((128, EC * SP), TR, tag="att", pool=app)
                for qh in range(2):
                    for p in range(4):
                        hg = p // 2
                        j0 = (p % 2) * 2
                        pab = pt((128, 512), tag="ab")
                        for tcb in range(8):
                            psc = pt((128, 1024), tag="sc")
                            for jj in range(2):
                                r0 = 32 * (j0 + jj)
                                MM(psc[:, jj * 512:(jj + 1) * 512],
                                   kT[r0:r0 + 32, hg * SP + tcb * 128: hg * SP + (tcb + 1) * 128],
                                   qTs[r0:r0 + 32, hg * SP + qh * 512: hg * SP + qh * 512 + 512],
                                   start=True, stop=True, tile_position=(r0, 0))
                            ee = app.tile([128, 1024], TB, tag="expE", bufs=2, name=_nm("ee"))
                            nc.scalar.activation(ee[:], psc[:], AF.Exp)
                            for jj in range(2):
                                hh = 4 * hg + j0 + jj
                                cg = 64 * jj
                                MM(pab[cg:cg + 33, :],
                                   Vtm[:, tcb * 264 + hh * 33: tcb * 264 + (hh + 1) * 33],
                                   ee[:, jj * 512:(jj + 1) * 512],
                                   start=(tcb == 0), stop=(tcb == 7), tile_position=(0, cg))
                        # normalize pair p
                        rcp = sm4.tile([1, 1024], TS, tag="rcp", bufs=1, name=_nm("s"))
                        uai = app.tile([128, 512], TS, tag="uai", bufs=2, name=_nm("u"))
                        copy(uai[:], pab[:])
                        for jj in range(2):
                            denr = sm.tile([1, 512], TS, tag="denr", bufs=2, name=_nm("s"))
                            copy(denr[0:1, 0:512], uai[64 * jj + 32: 64 * jj + 33, :])
                            nc.vector.reciprocal_approx_fast(
                                rcp[0:1, jj * 512:(jj + 1) * 512], denr[0:1, 0:512])
                        rcpr = sm.tile([1, 1024], TR, tag="rcpr", bufs=1, name=_nm("s"))
                        copy(rcpr[:], rcp[:])
                        for jj in range(2):
                            r0 = 32 * (j0 + jj)
                            prb = pt((32, 512))
                            MM(prb[:], o1x32r[:], rcpr[0:1, jj * 512:(jj + 1) * 512],
                               start=True, stop=True)
                            nc.vector.tensor_mul(
                                att[r0:r0 + 32, hg * SP + qh * 512: hg * SP + qh * 512 + 512],
                                uai[64 * jj: 64 * jj + 32, :], prb[:])
                # ---- tail: O-proj + residual ----
                tw = wtiles.pop(("tail", l))
                wo, w1, w2, bo, b1, b2 = tw["wo"], tw["w1"], tw["w2"], tw["bo"], tw["b1"], tw["b2"]
                resid = sb((128, EC * SP), TR, tag="resid", pool=app)
                for co in range(EC):
                    for th in range(2):
                        po = pt((128, 512))
                        for ci in range(EC):
                            MM(po[:, 0:500],
                               wo[:, (ci * EC + co) * 128:(ci * EC + co + 1) * 128],
                               att[:, ci * SP + th * 500: ci * SP + (th + 1) * 500],
                               start=(ci == 0), stop=(ci == EC - 1))
                        sl = resid[:, co * SP + th * 500: co * SP + (th + 1) * 500]
                        nc.vector.scalar_tensor_tensor(
                            sl, po[:, 0:500], bo[:, co:co + 1],
                            x[:, co * SP + th * 500: co * SP + (th + 1) * 500].bitcast(TS),
                            ALU.add, ALU.add)
                    copy(resid[:, co * SP + 1000: co * SP + 1024], zeros128[:, 0:24])
                x = ln_T(resid, lng[:, l * EC:(l + 1) * EC], lnb[:, l * EC:(l + 1) * EC])

                # ---- FFN ----
                resid2 = sb((128, EC * SP), TR, tag="resid", pool=app)
                for th in range(2):
                    hR = sb((128, HC * 512), TR, tag="hR", pool=app)
                    for hc in range(HC):
                        pf_ = pt((128, 512))
                        for ci in range(EC):
                            MM(pf_[:, 0:500],
                               w1[:, (ci * HC + hc) * 128:(ci * HC + hc + 1) * 128],
                               x[:, ci * SP + th * 500: ci * SP + (th + 1) * 500],
                               start=(ci == 0), stop=(ci == EC - 1))
                        nc.scalar.activation(hR[:, hc * 512: hc * 512 + 500],
                                             pf_[:, 0:500], AF.Gelu, bias=b1[:, hc:hc + 1])
                    for co in range(EC):
                        p2_ = pt((128, 512))
                        for hc in range(HC):
                            MM(p2_[:, 0:500],
                               w2[:, (hc * EC + co) * 128:(hc * EC + co + 1) * 128],
                               hR[:, hc * 512: hc * 512 + 500],
                               start=(hc == 0), stop=(hc == HC - 1))
                        sl = resid2[:, co * SP + th * 500: co * SP + (th + 1) * 500]
                        nc.vector.scalar_tensor_tensor(
                            sl, p2_[:, 0:500], b2[:, co:co + 1],
                            x[:, co * SP + th * 500: co * SP + (th + 1) * 500].bitcast(TS),
                            ALU.add, ALU.add)
                for co in range(EC):
                    copy(resid2[:, co * SP + 1000: co * SP + 1024], zeros128[:, 0:24])
                x = ln_T(resid2, lng[:, l * EC:(l + 1) * EC], lnb[:, l * EC:(l + 1) * EC])

                if l + 1 < L:
                    qTs, kT, Vtm = qkv_issue(l + 1, x)

            # ================= pooling + classifier =================
            pcs = pt((1, 1024), tag="sc")
            for co in range(EC):
                for th in range(2):
                    MMs(pcs[:, th * 512: th * 512 + 500], ones128r[:],
                       x[:, co * SP + th * 500: co * SP + (th + 1) * 500],
                       start=(co == 0), stop=(co == EC - 1))
            pwacc = sb((1, 2), tag="pwacc")
            pwr = sm.tile([1, 1024], TR, tag="rcpr", bufs=1, name=_nm("s"))
            for th in range(2):
                nc.scalar.activation(pwr[:, th * 512: th * 512 + 500],
                                     pcs[:, th * 512: th * 512 + 500], AF.Exp,
                                     accum_out=pwacc[:, th:th + 1])
            tot = sb((1, 1))
            nc.vector.tensor_add(tot[:], pwacc[:, 0:1], pwacc[:, 1:2])
            rtot = sb((1, 1))
            nc.vector.reciprocal(rtot[:], tot[:])
            pooled = sb((128, EC), tag="pooled")
            for co in range(EC):
                ppw = pt((128, 1024), tag="sc")
                for th in range(2):
                    MMs(ppw[:, th * 512:(th + 1) * 512], o1x128r[:],
                       pwr[:, th * 512:(th + 1) * 512], start=True, stop=True)
                xw = sb((128, 1024))
                for th in range(2):
                    nc.vector.tensor_mul(xw[:, th * 512: th * 512 + 500],
                                         x[:, co * SP + th * 500: co * SP + (th + 1) * 500].bitcast(TS),
                                         ppw[:, th * 512: th * 512 + 500])
                copy(xw[:, 500:512], zeros128[:, 0:12])
                copy(xw[:, 1012:1024], zeros128[:, 0:12])
                nc.vector.tensor_reduce(pooled[:, co:co + 1], xw[:], AX.X, ALU.add)
            rtotr = sb((1, 1), TR)
            copy(rtotr[:], rtot[:])
            prt = pt((128, 1))
            MMs(prt[:], o1x128r[:], rtotr[:], start=True, stop=True)
            rtb = sb((128, 1))
            copy(rtb[:], prt[:])
            nc.vector.tensor_scalar(pooled[:], pooled[:], rtb[:, 0:1], None, ALU.mult)
            # LN over the 256-vector
            poor = sb((128, EC), TR, tag="poor")
            copy(poor[:], pooled[:])
            poo2 = sb((128, EC), TR, tag="poo2")
            nc.vector.tensor_mul(poo2[:], pooled[:], pooled[:])
            pcs2 = pt((EC, 2))
            MMs(pcs2[:, 0:1], poor[:], ones128r[:], start=True, stop=True)
            MMs(pcs2[:, 1:2], poo2[:], ones128r[:], start=True, stop=True)
            cs2 = sb((EC, 2), TR)
            copy(cs2[:], pcs2[:])
            pcs3 = pt((2, 1))
            MMs(pcs3[:], cs2[:], onesECr[:], start=True, stop=True)
            cs3t = sb((2, 1), TR)
            copy(cs3t[:], pcs3[:])
            pcs4 = pt((1, 2))
            MMs(pcs4[:], cs3t[:], I4r[0:2, 0:2], start=True, stop=True)
            cs3 = sb((1, 2))
            nc.vector.tensor_scalar_mul(cs3[:], pcs4[:], 1.0 / 256.0)
            cm2 = sb((1, 1))
            nc.vector.tensor_mul(cm2[:], cs3[0:1, 0:1], cs3[0:1, 0:1])
            cvar = sb((1, 1))
            nc.vector.tensor_sub(cvar[:], cs3[0:1, 1:2], cm2[:])
            crstd = sb((1, 1), TR)
            rsqrt_row(crstd[:].bitcast(TS), cvar[:], bias=epsb[0:1, :])
            cmeanr = sb((1, 1), TR)
            copy(cmeanr[:], cs3[0:1, 0:1])
            pcb = pt((128, 2))
            MMs(pcb[:, 0:1], o1x128r[:], cmeanr[:], start=True, stop=True)
            MMs(pcb[:, 1:2], o1x128r[:], crstd[:], start=True, stop=True)
            yv = sb((128, EC), TR, tag="yv")
            for co in range(EC):
                t_ = sb((128, 1))
                nc.vector.tensor_sub(t_[:], pooled[:, co:co + 1], pcb[:, 0:1])
                nc.vector.tensor_scalar(t_[:], t_[:], pcb[:, 1:2], None, ALU.mult)
                nc.vector.tensor_scalar(yv[:, co:co + 1], t_[:], clng[:, co:co + 1],
                                        clnb[:, co:co + 1], ALU.mult, ALU.add)
            pz = pt((128, 1))
            for co in range(EC):
                MMs(pz[:], cw1T[:, co * 128:(co + 1) * 128], yv[:, co:co + 1],
                   start=(co == 0), stop=(co == EC - 1))
            zv = sb((128, 1), TR)
            nc.vector.tensor_scalar(zv[:], pz[:], cb1[:], None, ALU.add)
            nc.vector.tensor_scalar_max(zv[:].bitcast(TS), zv[:].bitcast(TS), 0.0)
            zv2 = sb((128, 1), TR)
            copy(zv2[:], zv[:].bitcast(TS))
            pout = pt((NCls, 1))
            MMs(pout[:], cw2T[:], zv2[:], start=True, stop=True)
            outv = sb((NCls, 1))
            nc.vector.tensor_scalar(outv[:], pout[:], cb2[:], None, ALU.add)
            nc.sync.dma_start(out_d, outv[:])

    nc.compile()
    return nc


_NC_CACHE = {}


def _host_inputs(inputs):
    I = {k: np.asarray(v, F32) for k, v in inputs.items()}
    h = {}
    h["embw1T"] = np.ascontiguousarray(I["emb_w1"].T)                       # [5,128]
    h["embb1"] = I["emb_b1"].reshape(128, 1)

    def wT(w):
        O, II = w.shape
        return np.ascontiguousarray(w.T.reshape(II // 128, 128, O).transpose(1, 0, 2)).reshape(128, -1)

    h["embw2T"] = wT(I["emb_w2"])                                           # [128, 256]
    h["embb2"] = np.ascontiguousarray(I["emb_b2"].reshape(EC, 128).T)
    posT = np.zeros((128, EC * SP), F32)
    pe = I["pos_enc"][:S]                                                   # [1000, 256]
    for co in range(EC):
        posT[:, co * SP: co * SP + S] = pe[:, co * 128:(co + 1) * 128].T
    h["posT"] = posT
    for nm_, key in (("qwT", "qw"), ("kwT", "kw"), ("vwT", "vw"), ("owT", "ow")):
        h[nm_] = np.concatenate([wT(I[key][l]) for l in range(L)], axis=1)
    h["qkvb"] = np.concatenate(
        [np.concatenate([I["qb"][l].reshape(EC, 128).T, I["kb"][l].reshape(EC, 128).T,
                         I["vb"][l].reshape(EC, 128).T], axis=1) for l in range(L)], axis=1)
    h["obias"] = np.concatenate([I["ob"][l].reshape(EC, 128).T for l in range(L)], axis=1)
    h["vbrow"] = I["vb"].reshape(1, L * E)
    h["fw1T"] = np.concatenate([wT(I["f_w1"][l]) for l in range(L)], axis=1)
    h["fw2T"] = np.concatenate([wT(I["f_w2"][l]) for l in range(L)], axis=1)
    h["fb1"] = np.concatenate([I["f_b1"][l].reshape(HC, 128).T for l in range(L)], axis=1)
    h["fb2"] = np.concatenate([I["f_b2"][l].reshape(EC, 128).T for l in range(L)], axis=1)
    h["lng"] = np.concatenate([I["ln_g"][l].reshape(EC, 128).T for l in range(L)], axis=1)
    h["lnb"] = np.concatenate([I["ln_b"][l].reshape(EC, 128).T for l in range(L)], axis=1)
    h["embln"] = np.concatenate([I["emb_ln_g"].reshape(EC, 128).T,
                                 I["emb_ln_b"].reshape(EC, 128).T], axis=1)
    h["pew1T"] = np.concatenate([np.ascontiguousarray(I["pe_w1"][l].T) for l in range(L)], axis=1)
    h["peb1"] = np.stack([I["pe_b1"][l] for l in range(L)], axis=1)
    h["pew2T"] = np.concatenate([wT(I["pe_w2"][l]) for l in range(L)], axis=1)
    h["peb2"] = np.concatenate([I["pe_b2"][l].reshape(EC, 128).T for l in range(L)], axis=1)
    h["clng"] = np.ascontiguousarray(I["c_ln_g"].reshape(EC, 128).T)
    h["clnb"] = np.ascontiguousarray(I["c_ln_b"].reshape(EC, 128).T)
    h["cw1T"] = wT(I["c_w1"])
    h["cb1"] = I["c_b1"].reshape(128, 1)
    h["cw2T"] = np.ascontiguousarray(I["c_w2"].T)                           # [128, 2]
    h["cb2"] = I["c_b2"].reshape(NCls, 1)
    h["ph_law"] = I["ph_law"].reshape(1, 2); h["ph_lab"] = I["ph_lab"].reshape(1, 1)
    h["ph_fw"] = I["ph_fw"].reshape(1, 1); h["ph_db"] = I["ph_db"].reshape(1, 1)
    tc_ = (np.arange(8)[None, :] * 128 + np.arange(128)[:, None]).astype(F32)
    h["tconst"] = tc_
    h["padneg"] = np.where(tc_ < S, F32(0), F32(-3e38)).astype(F32)
    h["iota50"] = np.broadcast_to(np.arange(50, dtype=F32), (128, 50)).copy()
    h["I50"] = np.eye(50, dtype=F32); h["maskD50"] = np.eye(50, dtype=F32)
    h["I4"] = np.eye(4, dtype=F32); h["I128"] = np.eye(128, dtype=F32)
    h["ones128"] = np.ones((128, 1), F32); h["ones50"] = np.ones((50, 1), F32)
    h["ones1x128"] = np.ones((1, 128), F32); h["ones1x50"] = np.ones((1, 50), F32)
    h["ones1x32"] = np.ones((1, 32), F32); h["ones4"] = np.ones((4, 1), F32)
    h["ones1x512"] = np.ones((1, 512), F32)
    h["onesEC"] = np.ones((EC, 1), F32)
    h["oneso256"] = np.full((128, 1), 1.0 / 256.0, F32)
    v0 = np.full((50, 1), 0.1414, F32); v0[::2, 0] *= -1
    h["v0"] = v0
    h["W0"] = (np.random.default_rng(1234).standard_normal((50, 4)).astype(F32) * F32(0.14))
    h["zeros128"] = np.zeros((128, 256), F32)
    h["epsb"] = np.full((128, 1), 1e-5, F32)
    vm = (tc_ < S).astype(F32)
    h["vmask8"] = np.repeat(vm, 8, axis=1)
    return h


def kernel(**inputs):
    if "nc" not in _NC_CACHE:
        _NC_CACHE["nc"] = build_nc()
    nc = _NC_CACHE["nc"]
    h = _host_inputs(inputs)
    seqs = np.asarray(inputs["sequences"], F32)
    in_maps = []
    for b in range(4):
        m = dict(h)
        seqp = np.zeros((SP, 5), F32)
        seqp[:S] = seqs[b]
        m["seqT5"] = np.ascontiguousarray(seqp.T)
        m["seqPH"] = np.ascontiguousarray(
            seqp.reshape(8, 128, 5).transpose(1, 0, 2).reshape(128, 40))
        in_maps.append(m)
    res = bass_utils.run_bass_kernel_spmd(nc, in_maps, core_ids=[0, 1, 2, 3])
    out = np.stack([res.results[b]["out"][:, 0] for b in range(4)], axis=0)
    return out.astype(np.float32)
